# revision 1
# baseline (speedup 1.0000x reference)
"""Self-contained TRN2 Bass kernel for nn_MultiHeadAttn_91010357002583.

Multi-head attention (B=4, S=2048, D=1024, H=16, hd=64), eval mode,
mask all-ones, char_ids/seq_len unused by the reference.

Sharding: 8 cores = 4 batches x 2 query-row halves. Each core:
  - receives x^T (bf16) for its batch with ITS query half's rows FIRST
    (attention is permutation-invariant over key rows, so reordering
    k rows is free; q rows stay in original order within the half);
  - computes full K^T / V for the batch (2x redundant) + Q^T for its half;
  - flash-style attention with scores transposed [k, q], softmax
    denominator fused as a col-tiled all-ones stationary matmul;
  - fc projection on its disjoint 1024 output rows.
Output is a pure concatenation — no collectives, no host reduction.
"""

import math
import sys
from contextlib import ExitStack

import numpy as np
import ml_dtypes

for _p in ("/opt/trn_rl_repo", "/root/.axon_site/_ro/trn_rl_repo"):
    if _p not in sys.path:
        sys.path.insert(0, _p)

import concourse.bass as bass  # noqa: E402
import concourse.tile as tile  # noqa: E402
from concourse import bacc, mybir  # noqa: E402
from concourse.bass_utils import run_bass_kernel_spmd  # noqa: E402

bf16 = ml_dtypes.bfloat16
FP32 = mybir.dt.float32
BF16 = mybir.dt.bfloat16
AF = mybir.ActivationFunctionType

B, S, D, H = 4, 2048, 1024, 16
HD = D // H
SCALE = math.sqrt(HD)


class Cfg:
    def __init__(self, R=2048, Q=1024, Hn=16, D=1024, repeats=1):
        assert R % 128 == 0 and Q % 128 == 0 and Hn % 4 == 0
        self.R, self.Q, self.Hn, self.D = R, Q, Hn, D
        self.FT = D // 128          # feature tiles (proj contraction)
        self.NCT = Hn // 2          # coltiles for Q (and K) = heads/2
        self.NRT = R // 128         # k row tiles
        self.NG = Hn // 4           # head groups of 4
        self.NJ = Hn * 64 // 128    # d-tiles for fc contraction
        self.NQT = Q // 128
        self.NRC = max(1, R // 1024)
        self.repeats = repeats
        self.scale = 1.0 / math.sqrt(64.0)


def build_nc(cfg: Cfg, num_devices=8):
    R, Q, Hn, Dm, FT = cfg.R, cfg.Q, cfg.Hn, cfg.D, cfg.FT
    nc = bacc.Bacc("TRN2", target_bir_lowering=False, debug=False,
                   enable_asserts=False, num_devices=num_devices)
    xt_d = nc.dram_tensor("xt", [Dm, R], BF16, kind="ExternalInput").ap()
    wqk_d = nc.dram_tensor("wqk", [Hn, 128, FT, 128], BF16,
                           kind="ExternalInput").ap()
    wv_d = nc.dram_tensor("wv", [FT, 128, Hn * 64], BF16,
                          kind="ExternalInput").ap()
    wfc_d = nc.dram_tensor("wfc", [cfg.NJ, 128, Dm], BF16,
                           kind="ExternalInput").ap()
    bfc_d = nc.dram_tensor("bfc", [128, Dm], FP32, kind="ExternalInput").ap()
    y_d = nc.dram_tensor("y", [Q, Dm], FP32, kind="ExternalOutput").ap()
    with tile.TileContext(nc) as tc:
        with ExitStack() as ctx:
            build_body(ctx, tc, cfg, xt_d, wqk_d, wv_d, wfc_d, bfc_d, y_d)
    nc.finalize()
    return nc


def build_body(ctx, tc, cfg: Cfg, xt_d, wqk_d, wv_d, wfc_d, bfc_d, y_d):
    nc = tc.nc
    R, Q, Hn, Dm, FT = cfg.R, cfg.Q, cfg.Hn, cfg.D, cfg.FT
    NCT, NRT, NG, NJ, NQT, NRC = (cfg.NCT, cfg.NRT, cfg.NG, cfg.NJ,
                                  cfg.NQT, cfg.NRC)

    persist = ctx.enter_context(tc.tile_pool(name="persist", bufs=1))
    wqk_pool = ctx.enter_context(tc.tile_pool(name="wqk", bufs=6))
    wv_pool = ctx.enter_context(tc.tile_pool(name="wv", bufs=2))
    attn_pool = ctx.enter_context(tc.tile_pool(name="attn", bufs=4))
    ysb_pool = ctx.enter_context(tc.tile_pool(name="ysb", bufs=2))
    spool = ctx.enter_context(tc.tile_pool(name="ps_s", bufs=2, space="PSUM"))
    opool = ctx.enter_context(tc.tile_pool(name="ps_o", bufs=2, space="PSUM"))

    for _rep in range(cfg.repeats):
        xt_sb = persist.tile([128, FT, R], BF16, tag="xt")
        KT_sb = persist.tile([128, NCT, R], BF16, tag="kt")
        QT_sb = persist.tile([128, NCT, Q], BF16, tag="qt")
        V_sb = persist.tile([128, NRT, Hn, 64], BF16, tag="v")
        ones_sb = persist.tile([128, 64], BF16, tag="ones")
        OT_sb = persist.tile([128, NJ, Q], BF16, tag="ot")
        wfc_sb = persist.tile([128, NJ, Dm], BF16, tag="wfc")
        bfc_sb = persist.tile([128, Dm], FP32, tag="bfc")

        for ft in range(FT):
            nc.sync.dma_start(xt_sb[:, ft, :], xt_d[ft * 128:(ft + 1) * 128, :])
        nc.sync.dma_start(wfc_sb[:], wfc_d.rearrange("j p d -> p j d"))
        nc.sync.dma_start(bfc_sb[:], bfc_d[:])
        nc.vector.memset(ones_sb[:], 1.0)

        wqk_tiles = {}

        def load_wqk(j):
            t = wqk_pool.tile([128, FT, 128], BF16, tag="wqk", name="wqk_t")
            nc.sync.dma_start(t[:], wqk_d[j])
            wqk_tiles[j] = t

        for g in range(NG):
            cts = [2 * g, 2 * g + 1]
            wv_sb = wv_pool.tile([128, FT, 256], BF16, tag="wv", name="wv_t")
            nc.sync.dma_start(
                wv_sb[:],
                wv_d[:, :, g * 256:(g + 1) * 256].rearrange("f p c -> p f c"))
            for ct in cts:
                load_wqk(ct)
                load_wqk(NCT + ct)
            # Q^T [col, qrow]
            for ct in cts:
                for qc in range(max(1, Q // 1024)):
                    qn = min(1024, Q)
                    ps = spool.tile([128, 1024], FP32, tag="ps_s",
                                    name="ps_q")[:, :qn]
                    for sc in range(0, qn, 512):
                        sn = min(512, qn - sc)
                        for ft in range(FT):
                            nc.tensor.matmul(
                                ps[:, sc:sc + sn], wqk_tiles[ct][:, ft, :],
                                xt_sb[:, ft, qc * 1024 + sc:qc * 1024 + sc + sn],
                                start=(ft == 0), stop=(ft == FT - 1))
                    nc.vector.tensor_copy(
                        QT_sb[:, ct, qc * 1024:qc * 1024 + qn], ps)
            # K^T [col, krow]
            for ct in cts:
                for rc in range(NRC):
                    rn = min(1024, R)
                    ps = spool.tile([128, 1024], FP32, tag="ps_s",
                                    name="ps_k")[:, :rn]
                    for sc in range(0, rn, 512):
                        sn = min(512, rn - sc)
                        for ft in range(FT):
                            nc.tensor.matmul(
                                ps[:, sc:sc + sn],
                                wqk_tiles[NCT + ct][:, ft, :],
                                xt_sb[:, ft, rc * 1024 + sc:rc * 1024 + sc + sn],
                                start=(ft == 0), stop=(ft == FT - 1))
                    nc.vector.tensor_copy(
                        KT_sb[:, ct, rc * 1024:rc * 1024 + rn], ps)
            # V natural [krow, col-block of 4 heads]
            for rt in range(NRT):
                ps = spool.tile([128, 1024], FP32, tag="ps_s",
                                name="ps_v")[:, :256]
                for ft in range(FT):
                    nc.tensor.matmul(
                        ps, xt_sb[:, ft, rt * 128:(rt + 1) * 128],
                        wv_sb[:, ft, :],
                        start=(ft == 0), stop=(ft == FT - 1))
                nc.vector.tensor_copy(
                    V_sb[:, rt, 4 * g:4 * g + 4, :],
                    ps.rearrange("p (h c) -> p h c", c=64))

            # attention for this group's 2 head pairs
            for hp in cts:
                oA = opool.tile([128, 1024], FP32, tag="ps_o",
                                name="oA")[:, :Q]
                oB = opool.tile([128, 1024], FP32, tag="ps_o",
                                name="oB")[:, :Q]
                for kt in range(NRT):
                    psA = spool.tile([128, 1024], FP32, tag="ps_s",
                                     name="psA")[:, :Q]
                    psB = spool.tile([128, 1024], FP32, tag="ps_s",
                                     name="psB")[:, :Q]
                    for sc in range(0, Q, 512):
                        sn = min(512, Q - sc)
                        nc.tensor.matmul(
                            psA[:, sc:sc + sn],
                            KT_sb[0:64, hp, kt * 128:(kt + 1) * 128],
                            QT_sb[0:64, hp, sc:sc + sn],
                            start=True, stop=True)
                        nc.tensor.matmul(
                            psB[:, sc:sc + sn],
                            KT_sb[64:128, hp, kt * 128:(kt + 1) * 128],
                            QT_sb[64:128, hp, sc:sc + sn],
                            start=True, stop=True)
                    aA = attn_pool.tile([128, Q], BF16, tag="aT", name="aA")
                    aB = attn_pool.tile([128, Q], BF16, tag="aT", name="aB")
                    nc.scalar.activation(aA[:], psA, AF.Exp, scale=cfg.scale)
                    nc.scalar.activation(aB[:], psB, AF.Exp, scale=cfg.scale)
                    st, sp = (kt == 0), (kt == NRT - 1)
                    for sc in range(0, Q, 512):
                        sn = min(512, Q - sc)
                        s_ = slice(sc, sc + sn)
                        nc.tensor.matmul(oA[0:64, s_], V_sb[:, kt, 2 * hp, :],
                                         aA[:, s_], start=st, stop=sp)
                        nc.tensor.matmul(oA[64:128, s_], ones_sb[:],
                                         aA[:, s_], start=st, stop=sp)
                        nc.tensor.matmul(oB[0:64, s_],
                                         V_sb[:, kt, 2 * hp + 1, :],
                                         aB[:, s_], start=st, stop=sp)
                        nc.tensor.matmul(oB[64:128, s_], ones_sb[:],
                                         aB[:, s_], start=st, stop=sp)
                denA = attn_pool.tile([64, 1024], FP32, tag="den",
                                      name="denA")[:, :Q]
                denB = attn_pool.tile([64, 1024], FP32, tag="den",
                                      name="denB")[:, :Q]
                nc.vector.reciprocal(denA, oA[64:128, :])
                nc.vector.reciprocal(denB, oB[64:128, :])
                nc.vector.tensor_mul(OT_sb[0:64, hp, :], oA[0:64, :], denA)
                nc.vector.tensor_mul(OT_sb[64:128, hp, :], oB[0:64, :], denB)

        # fc projection
        for qt in range(NQT):
            ps = spool.tile([128, 1024], FP32, tag="ps_s",
                            name="ps_fc")[:, :Dm]
            for cc in range(0, Dm, 512):
                for j in range(NJ):
                    nc.tensor.matmul(
                        ps[:, cc:cc + 512],
                        OT_sb[:, j, qt * 128:(qt + 1) * 128],
                        wfc_sb[:, j, cc:cc + 512],
                        start=(j == 0), stop=(j == NJ - 1))
            yt = ysb_pool.tile([128, Dm], FP32, tag="y", name="yt")
            nc.vector.tensor_add(yt[:], ps, bfc_sb[:])
            nc.sync.dma_start(y_d[qt * 128:(qt + 1) * 128, :], yt[:])


# ---------------- host side ----------------

def prep_core_inputs(cfg: Cfg, xb_perm, W_qkv, W_fc, b_fc):
    """xb_perm: [R, D] f32, rows already permuted (this core's q rows first)."""
    Dm, Hn, FT, NCT, NJ = cfg.D, cfg.Hn, cfg.FT, cfg.NCT, cfg.NJ
    xt = np.ascontiguousarray(xb_perm.T).astype(bf16)
    Wq = W_qkv[:, :NCT * 128]
    Wk = W_qkv[:, Dm:Dm + NCT * 128]
    Wv = W_qkv[:, 2 * Dm:2 * Dm + Hn * 64]
    wq_t = Wq.reshape(FT, 128, NCT, 128).transpose(2, 1, 0, 3)
    wk_t = Wk.reshape(FT, 128, NCT, 128).transpose(2, 1, 0, 3)
    wqk = np.ascontiguousarray(
        np.concatenate([wq_t, wk_t], axis=0)).astype(bf16)
    wv = np.ascontiguousarray(Wv.reshape(FT, 128, Hn * 64)).astype(bf16)
    wfc = np.ascontiguousarray(
        W_fc[:NJ * 128].reshape(NJ, 128, Dm)).astype(bf16)
    bfc = np.ascontiguousarray(
        np.broadcast_to(b_fc.astype(np.float32), (128, Dm)))
    return {"xt": xt, "wqk": wqk, "wv": wv, "wfc": wfc, "bfc": bfc}


_CACHE = {}


def _get_nc(repeats=1):
    key = ("nc", repeats)
    if key not in _CACHE:
        _CACHE[key] = build_nc(Cfg(R=S, Q=S // 2, Hn=H, D=D, repeats=repeats))
    return _CACHE[key]


def make_in_maps(x, W_qkv, W_fc, b_fc):
    cfg = Cfg(R=S, Q=S // 2, Hn=H, D=D)
    x = np.asarray(x, dtype=np.float32)
    in_maps = []
    for c in range(8):
        b, half = divmod(c, 2)
        r0 = half * (S // 2)
        order = np.concatenate([
            np.arange(r0, r0 + S // 2),
            np.arange(0, r0),
            np.arange(r0 + S // 2, S),
        ])
        xb = x[b][order]
        in_maps.append(prep_core_inputs(
            cfg, xb, np.asarray(W_qkv, np.float32),
            np.asarray(W_fc, np.float32), np.asarray(b_fc, np.float32)))
    return in_maps


def kernel(x, char_ids, seq_len, mask, W_qkv, W_fc, b_fc):
    """Full inputs in, full [B, S, D] float32 output out."""
    nc = _get_nc(repeats=1)
    in_maps = make_in_maps(x, W_qkv, W_fc, b_fc)
    res = run_bass_kernel_spmd(nc, in_maps, core_ids=list(range(8)))
    out = np.empty((B, S, D), dtype=np.float32)
    for c in range(8):
        b, half = divmod(c, 2)
        r0 = half * (S // 2)
        out[b, r0:r0 + S // 2, :] = res.results[c]["y"]
    return out
